# revision 84
# baseline (speedup 1.0000x reference)
"""Grouped per-sample MLP (conv1d groups=B) + GroupSwish + softmax, on 8 NeuronCores.

Data-parallel over the group/batch axis B=256: 32 groups per core,
processed as 8 quads of 4 groups stacked on the partition axis.

Per group g: h = W1[g] @ x[g] + b1[g]; GroupSwish; o = W2[g] @ h + b2[g];
softmax over the flattened [C*L] logits.

Design (fp8 DoubleRow + quad stacking + 3-stage software pipeline):
  - x and W1 cast to fp8 e4m3 host-side (end-to-end rel err 8.9e-3 vs the
    2e-2 gate, numpy-verified). W1 scaled by 16 to stay in e4m3 normal
    range; the 1/16 is folded into the activation scales. x pre-transposed
    host-side so each SBUF partition loads one contiguous 6KB run per
    group-pair (line-rate ~26GB/s per SDMA engine vs ~18 at 2KB).
  - W1 matmuls in fp8 DoubleRow perf mode: chunk pairs (K=256 per matmul)
    -> 3 DR matmuls per group, 13 matmuls per quad total.
  - DoubleRow may only write PSUM at base partition 0 (probed on HW), so
    lhsT is zero-padded per group: j0 [W1|0] / j1 [0|W1] 64-wide write
    h[0:64]; j2/j3 128-wide write h[0:128]. Costs 3x W1 bytes (still only
    15% of traffic); PE cost is unchanged (matmul time ~ moving free size).
  - PSUM start=True clears has_written for the WHOLE bank; to stay
    order-robust there is exactly ONE start=True matmul per quad: the
    block-diagonal K=16 remainder (writes all 128 partitions, its x/w
    stay resident in SBUF for all quads). Everything else accumulates.
  - h for 4 groups lives in one [128, 512] PSUM tile, so GroupSwish runs as
    ONE tanh-ACT + 2 DVE ops per quad (4x fewer instructions).
  - W2 is block-diagonalized host-side to [128, 40] bf16 per quad: ONE
    matmul produces all 4 groups' logits [40, 512]; softmax normalization
    via one [40,40] block-mask matmul + reciprocal + scale; 1/1.1 folded
    into W2; softplus(beta) and its products precomputed into a const blob.
  - GroupSwish via tanh (the only ACT table with both tanh and exp):
    (h+b1)*sigmoid(sp*(h+b1)) = ((h+b1)*0.5) * (1 + tanh(sp*(h+b1)/2)).
  - Softmax without max-subtraction (logits are O(1)).
  - 3-stage pipeline: stage1(q)=loads+W1+swish, stage1b(q-1)=W2+exp,
    stage2(q-2)=normalize+store, so every cross-engine dependency has a
    full quad of slack and never head-of-line-blocks the PE queue. Input
    DMA triggers are hoisted 3 quads ahead of use: the scalar(ACT) engine
    queue carries both activations and DMA triggers, and a trigger emitted
    after tanh(q) would starve the rings behind tanh's wait-for-h.
"""

import os
import numpy as np
import ml_dtypes
from contextlib import ExitStack

import concourse.mybir as mybir
import concourse.tile as tile
from concourse import bacc
from concourse.bass_utils import run_bass_kernel_spmd

B, X, Z, C, L = 256, 784, 32, 10, 512
NCORE = 8
GPC = B // NCORE  # 32 groups per core
NQ = GPC // 4  # 8 quads per core
P = 128
NCH = 6  # dense K-chunks of 128 (3 DoubleRow pairs); remainder 16 block-diag
KREM = X - NCH * P  # 16
W1SC = 16.0  # host-side W1 scale (fp8 normal range); folded into act scales
F32 = mybir.dt.float32
BF16 = mybir.dt.bfloat16
FP8 = mybir.dt.float8e4
BF16NP = ml_dtypes.bfloat16
FP8NP = mybir.dt.np(mybir.dt.float8e4)

# DoubleRow matmuls may only write PSUM at base partition 0 (probed:
# M32@0/M64@0/M128@0 pass walrus, anything at base 32/64 fails ISA checks).
# So lhsT is zero-padded per group so every DR write lands at base 0:
#   j0: [W1|0]  64-wide -> h[0:64]     j1: [0|W1]  64-wide -> h[0:64]
#   j2: [0,0,W1,0] 128-wide -> h[:]    j3: [0,0,0,W1] 128-wide -> h[:]
# per-pair block layout (pair i covers chunks 2i, 2i+1), offsets in elements:
# [j0: 2*64 @0][j1: 2*64 @128][j2: 2*128 @256][j3: 2*128 @512] = 768/pair
PAIRW = 768
J_OFF = (0, 128, 256, 512)
J_W = (64, 64, 128, 128)
WCOLS = 3 * PAIRW  # 2304

# const-blob column layout (f32, [128, CB_COLS])
CB_SPH = 0  # softplus(beta)/2/W1SC (tanh scale on h*W1SC), stacked [32j+z, q]
CB_SPB1 = NQ  # (softplus(beta)/2) * b1 (tanh bias)
CB_B1 = 2 * NQ  # W1SC * b1 (u path: (h16 + 16 b1) * (0.5/16))
CB_B2 = 3 * NQ  # b2 stacked [10j+c, q] (rows 0..40)
CB_BLK = 4 * NQ  # block mask [40, 40]: 1 if row//10 == col//10; tot40 = M@esum
CB_COLS = 4 * NQ + 40

DEFAULT_CFG = dict(
    w_engine="gpsimd",
    out_engine="gpsimd",
    const_engine="gpsimd",
    x_bufs=22,
    h_bufs=4,
    s_bufs=4,
    warmup=0,  # dummy matmuls during the startup DMA window: HAM un-throttles
    #             the PE clock (1.2->2.4GHz) after ~3.4us of sustained busy
    x_split=True,  # chunks 0-3 and 4-5 as separate DMAs for finer overlap
    debug=False,  # dump h and o per quad to DRAM
)

_CACHE: dict = {}


def _eng(nc, name):
    return getattr(nc, name)


def _build(cfg=DEFAULT_CFG):
    nc = bacc.Bacc("TRN2", target_bir_lowering=False, debug=False)

    # two groups packed per row so each partition loads one 6KB run
    xm = nc.dram_tensor(
        "xm", [GPC // 2, P, 2 * NCH * L], FP8, kind="ExternalInput"
    ).ap()
    # remainder x/w for ALL quads, loaded once at start and kept resident
    xr = nc.dram_tensor("xr", [4 * KREM, NQ * L], FP8, kind="ExternalInput").ap()
    # two quads packed per row -> 4.6KB DMA descriptors
    w1m = nc.dram_tensor(
        "w1m", [NQ // 2, P, 2 * WCOLS], FP8, kind="ExternalInput"
    ).ap()
    w1r = nc.dram_tensor("w1r", [4 * KREM, NQ * P], FP8, kind="ExternalInput").ap()
    w2b = nc.dram_tensor("w2b", [P, NQ * 4 * C], BF16, kind="ExternalInput").ap()
    cb = nc.dram_tensor("cb", [P, CB_COLS], F32, kind="ExternalInput").ap()
    out = nc.dram_tensor("out", [GPC, C, L], F32, kind="ExternalOutput").ap()
    if cfg.get("debug"):
        hdbg = nc.dram_tensor("hdbg", [NQ, P, L], F32, kind="ExternalOutput").ap()
        odbg = nc.dram_tensor("odbg", [NQ, 4 * C, L], F32, kind="ExternalOutput").ap()

    with tile.TileContext(nc) as tc, ExitStack() as ctx:
        consts = ctx.enter_context(tc.tile_pool(name="consts", bufs=1))
        xpool = ctx.enter_context(tc.tile_pool(name="x", bufs=cfg["x_bufs"]))
        wpool = ctx.enter_context(tc.tile_pool(name="w1", bufs=3))

        spool = ctx.enter_context(tc.tile_pool(name="act", bufs=cfg["s_bufs"]))
        hps = ctx.enter_context(
            tc.tile_pool(name="hps", bufs=cfg["h_bufs"], space="PSUM")
        )
        ops = ctx.enter_context(tc.tile_pool(name="ops", bufs=2, space="PSUM"))
        tps = ctx.enter_context(tc.tile_pool(name="tps", bufs=2, space="PSUM"))
        wps = ctx.enter_context(tc.tile_pool(name="wps", bufs=1, space="PSUM"))

        ce = _eng(nc, cfg["const_engine"])
        we = _eng(nc, cfg["w_engine"])
        oe = _eng(nc, cfg["out_engine"])

        w2t = consts.tile([P, NQ * 4 * C], BF16, name="w2t")
        ce.dma_start(w2t[:], w2b)
        cbt = consts.tile([P, CB_COLS], F32, name="cbt")
        ce.dma_start(cbt[:], cb)
        xrall = consts.tile([4 * KREM, NQ * L], FP8, name="xrall")
        nc.sync.dma_start(xrall[:], xr)
        wrall = consts.tile([4 * KREM, NQ * P], FP8, name="wrall")
        nc.scalar.dma_start(wrall[:], w1r)

        if cfg.get("warmup"):
            wz = consts.tile([1, L], BF16, name="wz")
            nc.vector.memset(wz[:], 0.0)
            wp = wps.tile([1, L], F32, name="wp")
            for _ in range(cfg["warmup"]):
                nc.tensor.matmul(
                    wp[:], wz[:, 0:1], wz[:], start=True, stop=True
                )

        # stage1b (W2+exp) for quad q is emitted after stage1 of quad q+1;
        # stage2 (softmax normalization) two quads behind. Cross-engine deps
        # (swish from DVE, exp accum from ACT, reciprocal from DVE) then
        # never head-of-line-block the PE queue.
        sws = {}  # q -> swish tile
        pend = {}  # q -> (expo, esum)

        def stage1b(q):
            sw = sws.pop(q)
            o = ops.tile([4 * C, L], F32, tag="o", name=f"o{q}")
            nc.tensor.matmul(
                o[:],
                w2t[:, q * 4 * C : (q + 1) * 4 * C],
                sw[:],
                start=True,
                stop=True,
            )
            expo = spool.tile([4 * C, L], F32, tag="expo", name=f"e{q}")
            esum = spool.tile([4 * C, 1], F32, tag="esum", name=f"es{q}")
            nc.scalar.activation(
                expo[:],
                o[:],
                mybir.ActivationFunctionType.Exp,
                bias=cbt[: 4 * C, CB_B2 + q : CB_B2 + q + 1],
                scale=1.0,
                accum_out=esum[:],
            )
            pend[q] = (expo, esum)

        def stage2(q):
            expo, esum = pend.pop(q)
            # one matmul: tot40[p] = sum of esum over p's group block
            tot = tps.tile([4 * C, 1], F32, tag="tot", name=f"tot{q}")
            nc.tensor.matmul(
                tot[:],
                cbt[: 4 * C, CB_BLK : CB_BLK + 4 * C],
                esum[:],
                start=True,
                stop=True,
            )
            inv = spool.tile([4 * C, 1], F32, tag="inv", name=f"inv{q}")
            nc.vector.reciprocal(inv[:], tot[:])
            res = spool.tile([4 * C, L], F32, tag="res", name=f"r{q}")
            nc.vector.tensor_scalar_mul(res[:], expo[:], inv[:, 0:1])
            nc.scalar.dma_start(
                out[4 * q : 4 * (q + 1)].rearrange("j c l -> (j c) l"),
                res[:],
            )

        # input DMA triggers are hoisted LOOKAHEAD quads ahead of their
        # consumers: the scalar (ACT) engine queue carries both activations
        # and DMA triggers, and a trigger emitted after tanh(q) would sit
        # behind tanh's wait-for-h, starving the DMA rings (head-of-line).
        LOOKAHEAD = 3
        wt2s = {}
        ins = {}  # q -> (wt, wr, xrt, xts)

        def emit_dmas(q):
            if q % 2 == 0:
                wt2 = wpool.tile([P, 2 * WCOLS], FP8, tag="wt", name=f"wt{q}")
                w_ring = nc.sync if q % 4 == 0 else nc.scalar
                w_ring.dma_start(wt2[:], w1m[q // 2])
                wt2s[q] = wt2
            wt = wt2s[q - q % 2][:, (q % 2) * WCOLS : (q % 2 + 1) * WCOLS]
            xts = []
            for pp in range(2):
                gp = 2 * q + pp
                xt = xpool.tile([P, 2 * NCH * L], FP8, tag="xt", name=f"xt{gp}")
                xe = nc.sync if (pp + q) % 2 == 0 else nc.scalar
                if q == 0:
                    # split quad 0's loads per group so the first DR matmuls
                    # start as soon as group 0's slice lands (~3us earlier)
                    e2 = nc.scalar if pp == 0 else nc.sync
                    xe.dma_start(xt[:, : NCH * L], xm[gp, :, : NCH * L])
                    e2.dma_start(xt[:, NCH * L :], xm[gp, :, NCH * L :])
                else:
                    xe.dma_start(xt[:], xm[gp])
                xts.append(xt)
            ins[q] = (wt, xts)

        for qq in range(min(LOOKAHEAD + 1, NQ)):
            emit_dmas(qq)
        for q in range(NQ):
            if q + LOOKAHEAD + 1 < NQ:
                emit_dmas(q + LOOKAHEAD + 1)
            wt, xts = ins.pop(q)

            # --- h[32j:32j+32] = W1SC * W1[g] @ x[g] for the quad ---
            # ONE start=True matmul per quad (the K=16 remainder, which
            # writes all 128 partitions); all DoubleRow matmuls accumulate.
            h = hps.tile([P, L], F32, tag="h", name=f"h{q}")
            nc.tensor.matmul(
                h[:],
                wrall[:, q * P : (q + 1) * P],
                xrall[:, q * L : (q + 1) * L],
                start=True,
                stop=False,
                skip_group_check=True,
            )
            for j in range(4):
                for i in range(3):
                    off, w = J_OFF[j], J_W[j]
                    lo = i * PAIRW + off
                    lhs = wt[:, lo : lo + 2 * w].rearrange("p (c m) -> p c m", c=2)
                    dst = h[0:w, :]
                    xb = (j % 2) * NCH * L
                    rhs = xts[j // 2][
                        :, xb + 2 * i * L : xb + 2 * (i + 1) * L
                    ].rearrange("p (c l) -> p c l", c=2)
                    nc.tensor.matmul(
                        dst,
                        lhs,
                        rhs,
                        start=False,
                        stop=(j == 3 and i == 2),
                        skip_group_check=True,
                        perf_mode=mybir.MatmulPerfMode.DoubleRow,
                    )

            if cfg.get("debug"):
                hcp = spool.tile([P, L], F32, tag="hcp", name=f"hcp{q}")
                nc.vector.tensor_copy(hcp[:], h[:])
                oe.dma_start(hdbg[q], hcp[:])

            # --- GroupSwish: ((h+b1)*0.5) * (1 + tanh(sp*(h+b1)/2)) ---
            # device h is W1SC*(W1@x); scales in cb fold the 1/W1SC back in
            t = spool.tile([P, L], F32, tag="t", name=f"t{q}")
            nc.scalar.activation(
                t[:],
                h[:],
                mybir.ActivationFunctionType.Tanh,
                bias=cbt[:, CB_SPB1 + q : CB_SPB1 + q + 1],
                scale=cbt[:, CB_SPH + q : CB_SPH + q + 1],
            )
            u = spool.tile([P, L], F32, tag="u", name=f"u{q}")
            nc.vector.tensor_scalar(
                u[:],
                h[:],
                cbt[:, CB_B1 + q : CB_B1 + q + 1],
                0.5 / W1SC,
                op0=mybir.AluOpType.add,
                op1=mybir.AluOpType.mult,
            )
            sw = spool.tile([P, L], BF16, tag="sw", name=f"sw{q}")
            nc.vector.scalar_tensor_tensor(
                sw[:],
                t[:],
                1.0,
                u[:],
                op0=mybir.AluOpType.add,
                op1=mybir.AluOpType.mult,
            )
            sws[q] = sw
            # W2+exp for quad q-1 (its swish has had a full quad of slack,
            # so W2 never head-of-line-blocks the PE behind a DVE wait);
            # normalization for quad q-2.
            if q >= 1:
                stage1b(q - 1)
            if q >= 2:
                stage2(q - 2)
        stage1b(NQ - 1)
        stage2(NQ - 2)
        stage2(NQ - 1)

    nc.compile()
    return nc


def _marshal(x, W1, b1, beta, W2, b2, cfg=DEFAULT_CFG):
    """Full inputs -> list of per-core input dicts (all heavy reshapes here)."""
    xg = np.ascontiguousarray(x, dtype=np.float32).reshape(B, X, L)
    # xm[gp, p, jj*NCH*L + c*L + l] = x[2gp+jj, 128c+p, l]
    xmain = (
        xg[:, : NCH * P]
        .reshape(B // 2, 2, NCH, P, L)
        .transpose(0, 3, 1, 2, 4)
        .astype(FP8NP)
        .reshape(B // 2, P, 2 * NCH * L)
    )
    # xrem[gq, 16j+r, l] = x[4gq+j, 768+r, l]
    xrem = xg[:, NCH * P :].astype(FP8NP).reshape(B // 4, 4 * KREM, L)

    w1s = W1.astype(np.float32, copy=False) * np.float32(W1SC)
    w1T = w1s.transpose(0, 2, 1)  # [B, X, Z]
    w1ck = w1T[:, : NCH * P].reshape(B // 4, 4, NCH, P, Z)  # [gq, j, c, p, z]
    w1m = np.zeros((B // 4, P, WCOLS), np.float32)
    for i in range(3):
        for cc in range(2):
            c = 2 * i + cc
            for j in range(4):
                base = i * PAIRW + J_OFF[j] + cc * J_W[j] + 32 * j
                w1m[:, :, base : base + Z] = w1ck[:, j, c]
    w1m = w1m.astype(FP8NP)
    # pack two quads per row: w1m2[q2, p, qq*WCOLS + c] = w1m[2*q2+qq, p, c]
    w1m = np.ascontiguousarray(
        w1m.reshape(B // 8, 2, P, WCOLS).transpose(0, 2, 1, 3)
    ).reshape(B // 8, P, 2 * WCOLS)
    # w1r[gq, 16j+r, 32j+z] = W1SC*W1T[4gq+j, 768+r, z], else 0 (block diag)
    w1r = np.zeros((B // 4, 4 * KREM, P), FP8NP)
    w1T4 = w1T.reshape(B // 4, 4, X, Z)
    for j in range(4):
        w1r[:, KREM * j : KREM * (j + 1), Z * j : Z * (j + 1)] = w1T4[
            :, j, NCH * P :
        ].astype(FP8NP)

    # w2b[gq, 32j+z, 10j+c] = W2[4gq+j, c, z]/1.1, else 0 (block diagonal)
    w2s = (W2.astype(np.float32, copy=False) * np.float32(1.0 / 1.1)).transpose(
        0, 2, 1
    )  # [B, Z, C]
    w2blk = np.zeros((B // 4, P, 4 * C), BF16NP)
    w2s4 = w2s.reshape(B // 4, 4, Z, C)
    for j in range(4):
        w2blk[:, Z * j : Z * (j + 1), C * j : C * (j + 1)] = w2s4[:, j].astype(
            BF16NP
        )

    # const blob per core [128, CB_COLS] f32
    b1f = b1.astype(np.float32, copy=False)
    b2f = b2.astype(np.float32, copy=False)
    sp = np.log1p(np.exp(beta.astype(np.float64))).astype(np.float32)  # softplus
    blk = np.zeros((P, 4 * C), np.float32)
    for j in range(4):
        blk[C * j : C * (j + 1), C * j : C * (j + 1)] = 1.0

    in_maps = []
    for core in range(NCORE):
        s = slice(core * GPC, (core + 1) * GPC)
        sq = slice(core * NQ, (core + 1) * NQ)
        cbc = np.zeros((P, CB_COLS), np.float32)
        # stacked [32j+z, q] views for this core's quads
        b1c = b1f[s].reshape(NQ, 4 * Z).T  # [128, NQ]
        spc = np.repeat(sp[s].reshape(NQ, 4), Z, axis=1).T * 0.5  # [128, NQ]
        cbc[:, CB_SPH : CB_SPH + NQ] = spc / np.float32(W1SC)
        cbc[:, CB_SPB1 : CB_SPB1 + NQ] = spc * b1c
        cbc[:, CB_B1 : CB_B1 + NQ] = b1c * np.float32(W1SC)
        cbc[: 4 * C, CB_B2 : CB_B2 + NQ] = b2f[s].reshape(NQ, 4 * C).T
        cbc[:, CB_BLK : CB_BLK + 4 * C] = blk
        sp2 = slice(core * GPC // 2, (core + 1) * GPC // 2)
        in_maps.append(
            {
                "xm": xmain[sp2],
                "xr": np.ascontiguousarray(xrem[sq].transpose(1, 0, 2)).reshape(
                    4 * KREM, NQ * L
                ),
                "w1m": w1m[core * NQ // 2 : (core + 1) * NQ // 2],
                "w1r": np.ascontiguousarray(w1r[sq].transpose(1, 0, 2)).reshape(
                    4 * KREM, NQ * P
                ),
                "w2b": np.ascontiguousarray(
                    w2blk[sq].transpose(1, 0, 2).reshape(P, NQ * 4 * C)
                ),
                "cb": cbc,
            }
        )
    return in_maps


def _run(in_maps, cfg=DEFAULT_CFG, trace=False, tmpdir=None):
    key = str(sorted(cfg.items()))
    if key not in _CACHE:
        _CACHE[key] = _build(cfg)
    return run_bass_kernel_spmd(
        _CACHE[key],
        in_maps,
        core_ids=list(range(NCORE)),
        trace=trace,
        tmpdir=tmpdir,
    )


_LAST = {}


def kernel(x, W1, b1, beta, W2, b2):
    in_maps = _marshal(x, W1, b1, beta, W2, b2)
    trace = bool(os.environ.get("KERNEL_TRACE"))
    r = _run(in_maps, trace=trace, tmpdir=os.environ.get("KERNEL_TRACE_DIR"))
    _LAST["results"] = r
    outs = [r.results[c]["out"].reshape(GPC, C * L) for c in range(NCORE)]
    return np.concatenate(outs, axis=0)


# revision 85
# speedup vs baseline: 1.0642x; 1.0642x over previous
"""Grouped per-sample MLP (conv1d groups=B) + GroupSwish + softmax, on 8 NeuronCores.

Data-parallel over the group/batch axis B=256: 32 groups per core,
processed as 8 quads of 4 groups stacked on the partition axis.

Per group g: h = W1[g] @ x[g] + b1[g]; GroupSwish; o = W2[g] @ h + b2[g];
softmax over the flattened [C*L] logits.

Design (fp8 DoubleRow + quad stacking + 3-stage software pipeline):
  - x and W1 cast to fp8 e4m3 host-side (end-to-end rel err 8.9e-3 vs the
    2e-2 gate, numpy-verified). W1 scaled by 16 to stay in e4m3 normal
    range; the 1/16 is folded into the activation scales. x pre-transposed
    host-side so each SBUF partition loads one contiguous 6KB run per
    group-pair (line-rate ~26GB/s per SDMA engine vs ~18 at 2KB).
  - W1 matmuls in fp8 DoubleRow perf mode: chunk pairs (K=256 per matmul)
    -> 3 DR matmuls per group, 13 matmuls per quad total.
  - DoubleRow may only write PSUM at base partition 0 (probed on HW), so
    lhsT is zero-padded per group: j0 [W1|0] / j1 [0|W1] 64-wide write
    h[0:64]; j2/j3 128-wide write h[0:128]. Costs 3x W1 bytes (still only
    15% of traffic); PE cost is unchanged (matmul time ~ moving free size).
  - PSUM start=True clears has_written for the WHOLE bank; to stay
    order-robust there is exactly ONE start=True matmul per quad: the
    block-diagonal K=16 remainder (writes all 128 partitions, its x/w
    stay resident in SBUF for all quads). Everything else accumulates.
  - h for 4 groups lives in one [128, 512] PSUM tile, so GroupSwish runs as
    ONE tanh-ACT + 2 DVE ops per quad (4x fewer instructions).
  - W2 is block-diagonalized host-side to [128, 40] bf16 per quad: ONE
    matmul produces all 4 groups' logits [40, 512]; softmax normalization
    via one [40,40] block-mask matmul + reciprocal + scale; 1/1.1 folded
    into W2; softplus(beta) and its products precomputed into a const blob.
  - GroupSwish via tanh (the only ACT table with both tanh and exp):
    (h+b1)*sigmoid(sp*(h+b1)) = ((h+b1)*0.5) * (1 + tanh(sp*(h+b1)/2)).
  - Softmax without max-subtraction (logits are O(1)).
  - 3-stage pipeline: stage1(q)=loads+W1+swish, stage1b(q-1)=W2+exp,
    stage2(q-2)=normalize+store, so every cross-engine dependency has a
    full quad of slack and never head-of-line-blocks the PE queue. Input
    DMA triggers are hoisted 3 quads ahead of use: the scalar(ACT) engine
    queue carries both activations and DMA triggers, and a trigger emitted
    after tanh(q) would starve the rings behind tanh's wait-for-h.
"""

import os
import numpy as np
import ml_dtypes
from contextlib import ExitStack

import concourse.mybir as mybir
import concourse.tile as tile
from concourse import bacc
from concourse.bass_utils import run_bass_kernel_spmd

B, X, Z, C, L = 256, 784, 32, 10, 512
NCORE = 8
GPC = B // NCORE  # 32 groups per core
NQ = GPC // 4  # 8 quads per core
P = 128
NCH = 6  # dense K-chunks of 128 (3 DoubleRow pairs); remainder 16 block-diag
KREM = X - NCH * P  # 16
W1SC = 16.0  # host-side W1 scale (fp8 normal range); folded into act scales
F32 = mybir.dt.float32
BF16 = mybir.dt.bfloat16
FP8 = mybir.dt.float8e4
BF16NP = ml_dtypes.bfloat16
FP8NP = mybir.dt.np(mybir.dt.float8e4)

# DoubleRow matmuls may only write PSUM at base partition 0 (probed:
# M32@0/M64@0/M128@0 pass walrus, anything at base 32/64 fails ISA checks).
# So lhsT is zero-padded per group so every DR write lands at base 0:
#   j0: [W1|0]  64-wide -> h[0:64]     j1: [0|W1]  64-wide -> h[0:64]
#   j2: [0,0,W1,0] 128-wide -> h[:]    j3: [0,0,0,W1] 128-wide -> h[:]
# per-pair block layout (pair i covers chunks 2i, 2i+1), offsets in elements:
# [j0: 2*64 @0][j1: 2*64 @128][j2: 2*128 @256][j3: 2*128 @512] = 768/pair
PAIRW = 768
J_OFF = (0, 128, 256, 512)
J_W = (64, 64, 128, 128)
WCOLS = 3 * PAIRW  # 2304

# const-blob column layout (f32, [128, CB_COLS])
CB_SPH = 0  # softplus(beta)/2/W1SC (tanh scale on h*W1SC), stacked [32j+z, q]
CB_SPB1 = NQ  # (softplus(beta)/2) * b1 (tanh bias)
CB_B1 = 2 * NQ  # W1SC * b1 (u path: (h16 + 16 b1) * (0.5/16))
CB_B2 = 3 * NQ  # b2 stacked [10j+c, q] (rows 0..40)
CB_BLK = 4 * NQ  # block mask [40, 40]: 1 if row//10 == col//10; tot40 = M@esum
CB_COLS = 4 * NQ + 40

DEFAULT_CFG = dict(
    w_engine="gpsimd",
    out_engine="gpsimd",
    const_engine="gpsimd",
    x_bufs=22,
    h_bufs=4,
    s_bufs=4,
    warmup=0,  # dummy matmuls during the startup DMA window: HAM un-throttles
    #             the PE clock (1.2->2.4GHz) after ~3.4us of sustained busy
    x_split=True,  # chunks 0-3 and 4-5 as separate DMAs for finer overlap
    debug=False,  # dump h and o per quad to DRAM
)

_CACHE: dict = {}


def _eng(nc, name):
    return getattr(nc, name)


def _build(cfg=DEFAULT_CFG):
    nc = bacc.Bacc("TRN2", target_bir_lowering=False, debug=False)

    # two groups packed per row so each partition loads one 6KB run
    xm = nc.dram_tensor(
        "xm", [GPC // 2, P, 2 * NCH * L], FP8, kind="ExternalInput"
    ).ap()
    # remainder x/w for ALL quads, loaded once at start and kept resident
    xr = nc.dram_tensor("xr", [4 * KREM, NQ * L], FP8, kind="ExternalInput").ap()
    # two quads packed per row -> 4.6KB DMA descriptors
    w1m = nc.dram_tensor(
        "w1m", [NQ // 2, P, 2 * WCOLS], FP8, kind="ExternalInput"
    ).ap()
    w1r = nc.dram_tensor("w1r", [4 * KREM, NQ * P], FP8, kind="ExternalInput").ap()
    w2b = nc.dram_tensor("w2b", [P, NQ * 4 * C], BF16, kind="ExternalInput").ap()
    cb = nc.dram_tensor("cb", [P, CB_COLS], F32, kind="ExternalInput").ap()
    out = nc.dram_tensor("out", [GPC, C, L], F32, kind="ExternalOutput").ap()
    if cfg.get("debug"):
        hdbg = nc.dram_tensor("hdbg", [NQ, P, L], F32, kind="ExternalOutput").ap()
        odbg = nc.dram_tensor("odbg", [NQ, 4 * C, L], F32, kind="ExternalOutput").ap()

    with tile.TileContext(nc) as tc, ExitStack() as ctx:
        consts = ctx.enter_context(tc.tile_pool(name="consts", bufs=1))
        xpool = ctx.enter_context(tc.tile_pool(name="x", bufs=cfg["x_bufs"]))
        wpool = ctx.enter_context(tc.tile_pool(name="w1", bufs=3))

        spool = ctx.enter_context(tc.tile_pool(name="act", bufs=cfg["s_bufs"]))
        hps = ctx.enter_context(
            tc.tile_pool(name="hps", bufs=cfg["h_bufs"], space="PSUM")
        )
        ops = ctx.enter_context(tc.tile_pool(name="ops", bufs=2, space="PSUM"))
        tps = ctx.enter_context(tc.tile_pool(name="tps", bufs=2, space="PSUM"))
        wps = ctx.enter_context(tc.tile_pool(name="wps", bufs=1, space="PSUM"))

        ce = _eng(nc, cfg["const_engine"])
        we = _eng(nc, cfg["w_engine"])
        oe = _eng(nc, cfg["out_engine"])

        w2t = consts.tile([P, NQ * 4 * C], BF16, name="w2t")
        ce.dma_start(w2t[:], w2b)
        cbt = consts.tile([P, CB_COLS], F32, name="cbt")
        ce.dma_start(cbt[:], cb)
        xrall = consts.tile([4 * KREM, NQ * L], FP8, name="xrall")
        nc.sync.dma_start(xrall[:], xr)
        wrall = consts.tile([4 * KREM, NQ * P], FP8, name="wrall")
        nc.scalar.dma_start(wrall[:], w1r)

        if cfg.get("warmup"):
            wz = consts.tile([1, L], BF16, name="wz")
            nc.vector.memset(wz[:], 0.0)
            wp = wps.tile([1, L], F32, name="wp")
            for _ in range(cfg["warmup"]):
                nc.tensor.matmul(
                    wp[:], wz[:, 0:1], wz[:], start=True, stop=True
                )

        # stage1b (W2+exp) for quad q is emitted after stage1 of quad q+1;
        # stage2 (softmax normalization) two quads behind. Cross-engine deps
        # (swish from DVE, exp accum from ACT, reciprocal from DVE) then
        # never head-of-line-block the PE queue.
        sws = {}  # q -> swish tile
        pend = {}  # q -> (expo, esum)

        def stage1b(q):
            sw = sws.pop(q)
            o = ops.tile([4 * C, L], F32, tag="o", name=f"o{q}")
            nc.tensor.matmul(
                o[:],
                w2t[:, q * 4 * C : (q + 1) * 4 * C],
                sw[:],
                start=True,
                stop=True,
            )
            expo = spool.tile([4 * C, L], F32, tag="expo", name=f"e{q}")
            esum = spool.tile([4 * C, 1], F32, tag="esum", name=f"es{q}")
            nc.scalar.activation(
                expo[:],
                o[:],
                mybir.ActivationFunctionType.Exp,
                bias=cbt[: 4 * C, CB_B2 + q : CB_B2 + q + 1],
                scale=1.0,
                accum_out=esum[:],
            )
            pend[q] = (expo, esum)

        def stage2(q):
            expo, esum = pend.pop(q)
            # one matmul: tot40[p] = sum of esum over p's group block
            tot = tps.tile([4 * C, 1], F32, tag="tot", name=f"tot{q}")
            nc.tensor.matmul(
                tot[:],
                cbt[: 4 * C, CB_BLK : CB_BLK + 4 * C],
                esum[:],
                start=True,
                stop=True,
            )
            inv = spool.tile([4 * C, 1], F32, tag="inv", name=f"inv{q}")
            nc.vector.reciprocal(inv[:], tot[:])
            res = spool.tile([4 * C, L], F32, tag="res", name=f"r{q}")
            nc.vector.tensor_scalar_mul(res[:], expo[:], inv[:, 0:1])
            nc.sync.dma_start(
                out[4 * q : 4 * (q + 1)].rearrange("j c l -> (j c) l"),
                res[:],
            )

        # input DMA triggers are hoisted LOOKAHEAD quads ahead of their
        # consumers: the scalar (ACT) engine queue carries both activations
        # and DMA triggers, and a trigger emitted after tanh(q) would sit
        # behind tanh's wait-for-h, starving the DMA rings (head-of-line).
        LOOKAHEAD = 3
        wt2s = {}
        ins = {}  # q -> (wt, wr, xrt, xts)

        def emit_dmas(q):
            if q % 2 == 0:
                wt2 = wpool.tile([P, 2 * WCOLS], FP8, tag="wt", name=f"wt{q}")
                w_ring = nc.sync if q % 4 == 0 else nc.scalar
                w_ring.dma_start(wt2[:], w1m[q // 2])
                wt2s[q] = wt2
            wt = wt2s[q - q % 2][:, (q % 2) * WCOLS : (q % 2 + 1) * WCOLS]
            xts = []
            for pp in range(2):
                gp = 2 * q + pp
                xt = xpool.tile([P, 2 * NCH * L], FP8, tag="xt", name=f"xt{gp}")
                xe = nc.sync if (pp + q) % 2 == 0 else nc.scalar
                if q == 0:
                    # split quad 0's loads per group so the first DR matmuls
                    # start as soon as group 0's slice lands (~3us earlier)
                    e2 = nc.scalar if pp == 0 else nc.sync
                    xe.dma_start(xt[:, : NCH * L], xm[gp, :, : NCH * L])
                    e2.dma_start(xt[:, NCH * L :], xm[gp, :, NCH * L :])
                else:
                    xe.dma_start(xt[:], xm[gp])
                xts.append(xt)
            ins[q] = (wt, xts)

        for qq in range(min(LOOKAHEAD + 1, NQ)):
            emit_dmas(qq)
        for q in range(NQ):
            if q + LOOKAHEAD + 1 < NQ:
                emit_dmas(q + LOOKAHEAD + 1)
            wt, xts = ins.pop(q)

            # --- h[32j:32j+32] = W1SC * W1[g] @ x[g] for the quad ---
            # ONE start=True matmul per quad (the K=16 remainder, which
            # writes all 128 partitions); all DoubleRow matmuls accumulate.
            h = hps.tile([P, L], F32, tag="h", name=f"h{q}")
            nc.tensor.matmul(
                h[:],
                wrall[:, q * P : (q + 1) * P],
                xrall[:, q * L : (q + 1) * L],
                start=True,
                stop=False,
                skip_group_check=True,
            )
            for j in range(4):
                for i in range(3):
                    off, w = J_OFF[j], J_W[j]
                    lo = i * PAIRW + off
                    lhs = wt[:, lo : lo + 2 * w].rearrange("p (c m) -> p c m", c=2)
                    dst = h[0:w, :]
                    xb = (j % 2) * NCH * L
                    rhs = xts[j // 2][
                        :, xb + 2 * i * L : xb + 2 * (i + 1) * L
                    ].rearrange("p (c l) -> p c l", c=2)
                    nc.tensor.matmul(
                        dst,
                        lhs,
                        rhs,
                        start=False,
                        stop=(j == 3 and i == 2),
                        skip_group_check=True,
                        perf_mode=mybir.MatmulPerfMode.DoubleRow,
                    )

            if cfg.get("debug"):
                hcp = spool.tile([P, L], F32, tag="hcp", name=f"hcp{q}")
                nc.vector.tensor_copy(hcp[:], h[:])
                oe.dma_start(hdbg[q], hcp[:])

            # --- GroupSwish: ((h+b1)*0.5) * (1 + tanh(sp*(h+b1)/2)) ---
            # device h is W1SC*(W1@x); scales in cb fold the 1/W1SC back in
            t = spool.tile([P, L], F32, tag="t", name=f"t{q}")
            nc.scalar.activation(
                t[:],
                h[:],
                mybir.ActivationFunctionType.Tanh,
                bias=cbt[:, CB_SPB1 + q : CB_SPB1 + q + 1],
                scale=cbt[:, CB_SPH + q : CB_SPH + q + 1],
            )
            u = spool.tile([P, L], F32, tag="u", name=f"u{q}")
            nc.vector.tensor_scalar(
                u[:],
                h[:],
                cbt[:, CB_B1 + q : CB_B1 + q + 1],
                0.5 / W1SC,
                op0=mybir.AluOpType.add,
                op1=mybir.AluOpType.mult,
            )
            sw = spool.tile([P, L], BF16, tag="sw", name=f"sw{q}")
            nc.vector.scalar_tensor_tensor(
                sw[:],
                t[:],
                1.0,
                u[:],
                op0=mybir.AluOpType.add,
                op1=mybir.AluOpType.mult,
            )
            sws[q] = sw
            # W2+exp for quad q-1 (its swish has had a full quad of slack,
            # so W2 never head-of-line-blocks the PE behind a DVE wait);
            # normalization for quad q-2.
            if q >= 1:
                stage1b(q - 1)
            if q >= 2:
                stage2(q - 2)
        stage1b(NQ - 1)
        stage2(NQ - 2)
        stage2(NQ - 1)

    nc.compile()
    return nc


def _marshal(x, W1, b1, beta, W2, b2, cfg=DEFAULT_CFG):
    """Full inputs -> list of per-core input dicts (all heavy reshapes here)."""
    xg = np.ascontiguousarray(x, dtype=np.float32).reshape(B, X, L)
    # xm[gp, p, jj*NCH*L + c*L + l] = x[2gp+jj, 128c+p, l]
    xmain = (
        xg[:, : NCH * P]
        .reshape(B // 2, 2, NCH, P, L)
        .transpose(0, 3, 1, 2, 4)
        .astype(FP8NP)
        .reshape(B // 2, P, 2 * NCH * L)
    )
    # xrem[gq, 16j+r, l] = x[4gq+j, 768+r, l]
    xrem = xg[:, NCH * P :].astype(FP8NP).reshape(B // 4, 4 * KREM, L)

    w1s = W1.astype(np.float32, copy=False) * np.float32(W1SC)
    w1T = w1s.transpose(0, 2, 1)  # [B, X, Z]
    w1ck = w1T[:, : NCH * P].reshape(B // 4, 4, NCH, P, Z)  # [gq, j, c, p, z]
    w1m = np.zeros((B // 4, P, WCOLS), np.float32)
    for i in range(3):
        for cc in range(2):
            c = 2 * i + cc
            for j in range(4):
                base = i * PAIRW + J_OFF[j] + cc * J_W[j] + 32 * j
                w1m[:, :, base : base + Z] = w1ck[:, j, c]
    w1m = w1m.astype(FP8NP)
    # pack two quads per row: w1m2[q2, p, qq*WCOLS + c] = w1m[2*q2+qq, p, c]
    w1m = np.ascontiguousarray(
        w1m.reshape(B // 8, 2, P, WCOLS).transpose(0, 2, 1, 3)
    ).reshape(B // 8, P, 2 * WCOLS)
    # w1r[gq, 16j+r, 32j+z] = W1SC*W1T[4gq+j, 768+r, z], else 0 (block diag)
    w1r = np.zeros((B // 4, 4 * KREM, P), FP8NP)
    w1T4 = w1T.reshape(B // 4, 4, X, Z)
    for j in range(4):
        w1r[:, KREM * j : KREM * (j + 1), Z * j : Z * (j + 1)] = w1T4[
            :, j, NCH * P :
        ].astype(FP8NP)

    # w2b[gq, 32j+z, 10j+c] = W2[4gq+j, c, z]/1.1, else 0 (block diagonal)
    w2s = (W2.astype(np.float32, copy=False) * np.float32(1.0 / 1.1)).transpose(
        0, 2, 1
    )  # [B, Z, C]
    w2blk = np.zeros((B // 4, P, 4 * C), BF16NP)
    w2s4 = w2s.reshape(B // 4, 4, Z, C)
    for j in range(4):
        w2blk[:, Z * j : Z * (j + 1), C * j : C * (j + 1)] = w2s4[:, j].astype(
            BF16NP
        )

    # const blob per core [128, CB_COLS] f32
    b1f = b1.astype(np.float32, copy=False)
    b2f = b2.astype(np.float32, copy=False)
    sp = np.log1p(np.exp(beta.astype(np.float64))).astype(np.float32)  # softplus
    blk = np.zeros((P, 4 * C), np.float32)
    for j in range(4):
        blk[C * j : C * (j + 1), C * j : C * (j + 1)] = 1.0

    in_maps = []
    for core in range(NCORE):
        s = slice(core * GPC, (core + 1) * GPC)
        sq = slice(core * NQ, (core + 1) * NQ)
        cbc = np.zeros((P, CB_COLS), np.float32)
        # stacked [32j+z, q] views for this core's quads
        b1c = b1f[s].reshape(NQ, 4 * Z).T  # [128, NQ]
        spc = np.repeat(sp[s].reshape(NQ, 4), Z, axis=1).T * 0.5  # [128, NQ]
        cbc[:, CB_SPH : CB_SPH + NQ] = spc / np.float32(W1SC)
        cbc[:, CB_SPB1 : CB_SPB1 + NQ] = spc * b1c
        cbc[:, CB_B1 : CB_B1 + NQ] = b1c * np.float32(W1SC)
        cbc[: 4 * C, CB_B2 : CB_B2 + NQ] = b2f[s].reshape(NQ, 4 * C).T
        cbc[:, CB_BLK : CB_BLK + 4 * C] = blk
        sp2 = slice(core * GPC // 2, (core + 1) * GPC // 2)
        in_maps.append(
            {
                "xm": xmain[sp2],
                "xr": np.ascontiguousarray(xrem[sq].transpose(1, 0, 2)).reshape(
                    4 * KREM, NQ * L
                ),
                "w1m": w1m[core * NQ // 2 : (core + 1) * NQ // 2],
                "w1r": np.ascontiguousarray(w1r[sq].transpose(1, 0, 2)).reshape(
                    4 * KREM, NQ * P
                ),
                "w2b": np.ascontiguousarray(
                    w2blk[sq].transpose(1, 0, 2).reshape(P, NQ * 4 * C)
                ),
                "cb": cbc,
            }
        )
    return in_maps


def _run(in_maps, cfg=DEFAULT_CFG, trace=False, tmpdir=None):
    key = str(sorted(cfg.items()))
    if key not in _CACHE:
        _CACHE[key] = _build(cfg)
    return run_bass_kernel_spmd(
        _CACHE[key],
        in_maps,
        core_ids=list(range(NCORE)),
        trace=trace,
        tmpdir=tmpdir,
    )


_LAST = {}


def kernel(x, W1, b1, beta, W2, b2):
    in_maps = _marshal(x, W1, b1, beta, W2, b2)
    trace = bool(os.environ.get("KERNEL_TRACE"))
    r = _run(in_maps, trace=trace, tmpdir=os.environ.get("KERNEL_TRACE_DIR"))
    _LAST["results"] = r
    outs = [r.results[c]["out"].reshape(GPC, C * L) for c in range(NCORE)]
    return np.concatenate(outs, axis=0)
